# revision 1
# baseline (speedup 1.0000x reference)
"""Trainium2 Bass kernel for nn_CandidateFilterModel (segment_reduce).

Strategy (8 cores, S-column sharding for the heavy phases, pair sharding for the tail):
  - Core k owns sequence-column slice s_k = [256k, 256k+256).
  - Phase 1: entity aggregation.
      ent_emb^T = log(OH_emb-matmul of exp(seq[mention_idx]))   (replicated, bf16 matmuls)
      ent_att (local s-slice) = OH_mean-matmul of gathered attention rows -> DRAM table
  - Phase 2: pair products. For all 2048 pairs: gather ent_att rows of head/tail
      entity (8KB bf16 rows, indirect DMA), multiply, tree-reduce over 16 heads ->
      RAW[p, s_local]; PE-transpose to RAW^T[s_local, p].
  - Phase 3: ONE AllToAll redistributes RAW^T so core k holds RAW^T[:, P_k] for
      its 256 pairs over ALL s. (This avoids an expensive 8MB AllReduce; measured
      AllToAll cost ~13us vs ~107us for the AllReduce.)
  - Phase 4: pairs-local tail with full weights:
      uc^T = [seq|1]^T-matmul -> contexts + Z row; normalize; z_s/z_o via
      (ent_emb @ W)-then-gather one-hot matmuls + W_ctx matmuls + tanh;
      bilinear via W_bil matmuls + elementwise + ones-reduction matmul.
All matmuls bf16 (fp32 PSUM accumulate): fp32 matmul measured ~3.6x slower on PE.
"""
import sys
import types
import numpy as np

S, H, HEADS = 2048, 1024, 16
E, NM, P = 256, 1024, 2048
PH = 1024
NC = 8
SL = S // NC          # 256 s-columns per core
PL = P // NC          # 256 pairs per core
NMT = NM // 128       # 8 mention tiles
NPT = P // 128        # 16 pair tiles

_CACHE = {}


def _ensure_axon_profile_hook():
    """bass_utils' trace path imports antenv.axon_hooks, absent in this image."""
    if 'antenv.axon_hooks' in sys.modules:
        return
    try:
        import antenv.axon_hooks  # noqa: F401
        return
    except ImportError:
        pass
    mod = types.ModuleType('antenv.axon_hooks')
    holder = [None]
    mod.set_axon_ntff_profile_hook = lambda h: holder.__setitem__(0, h)
    mod.get_axon_ntff_profile_hook = lambda: holder[0]
    sys.modules['antenv.axon_hooks'] = mod
    try:
        from trn_agent_boot.trn_boot import _ntff_profile_via_ctypes
        hook = _ntff_profile_via_ctypes('/opt/axon/libaxon_pjrt.so')
        if hook is not None:
            mod.set_axon_ntff_profile_hook(hook)
    except Exception:
        pass


def _build(debug=False):
    import concourse.bass as bass
    import concourse.bacc as bacc
    import concourse.tile as tile
    from concourse import mybir
    from concourse.masks import make_identity

    F32 = mybir.dt.float32
    BF16 = mybir.dt.bfloat16
    I32 = mybir.dt.int32
    AF = mybir.ActivationFunctionType
    OP = mybir.AluOpType

    nc = bacc.Bacc(num_devices=NC)

    # ---------------- inputs ----------------
    att_k = nc.declare_dram_parameter("att_k", [S, HEADS * SL], F32, isOutput=False)
    seq = nc.declare_dram_parameter("seq", [S, H], F32, isOutput=False)
    m_off = nc.declare_dram_parameter("m_off", [128, NMT], I32, isOutput=False)
    p_off = nc.declare_dram_parameter("p_off", [128, 2 * NPT], I32, isOutput=False)
    ohe = nc.declare_dram_parameter("ohe", [NM, E], BF16, isOutput=False)
    ohm = nc.declare_dram_parameter("ohm", [NM, E], BF16, isOutput=False)
    has0r = nc.declare_dram_parameter("has0r", [1, E], F32, isOutput=False)
    ohh_k = nc.declare_dram_parameter("ohh_k", [E, PL], BF16, isOutput=False)
    oht_k = nc.declare_dram_parameter("oht_k", [E, PL], BF16, isOutput=False)
    w_head = nc.declare_dram_parameter("w_head", [H, PH], F32, isOutput=False)
    w_tail = nc.declare_dram_parameter("w_tail", [H, PH], F32, isOutput=False)
    w_ctx = nc.declare_dram_parameter("w_ctx", [H, PH], F32, isOutput=False)
    w_bil = nc.declare_dram_parameter("w_bil", [PH, PH], F32, isOutput=False)
    b_head = nc.declare_dram_parameter("b_head", [128, PH // 128], F32, isOutput=False)
    b_tail = nc.declare_dram_parameter("b_tail", [128, PH // 128], F32, isOutput=False)
    b_bil = nc.declare_dram_parameter("b_bil", [1, 1], F32, isOutput=False)
    out = nc.declare_dram_parameter("out", [1, PL], F32, isOutput=True)

    dbg = {}
    if debug:
        dbg["ent_embT"] = nc.declare_dram_parameter("d_ent_embT", [H, E], mybir.dt.bfloat16, isOutput=True)
        dbg["entA"] = nc.declare_dram_parameter("d_entA", [E, HEADS * SL], mybir.dt.bfloat16, isOutput=True)
        dbg["raw"] = nc.declare_dram_parameter("d_raw", [128, NPT, SL], mybir.dt.bfloat16, isOutput=True)
        dbg["ctxuT"] = nc.declare_dram_parameter("d_ctxuT", [H, PL], mybir.dt.bfloat16, isOutput=True)
        dbg["zrow"] = nc.declare_dram_parameter("d_zrow", [1, PL], F32, isOutput=True)
        dbg["zsT"] = nc.declare_dram_parameter("d_zsT", [PH, PL], mybir.dt.bfloat16, isOutput=True)

    # internal DRAM
    entA_dram = nc.dram_tensor("entA_dram", [E, HEADS * SL], BF16)
    zrec_dram = nc.dram_tensor("zrec_dram", [1, PL], BF16)
    a2a_in = nc.dram_tensor("a2a_in", [NC, SL, PL], BF16)
    a2a_out = nc.dram_tensor("a2a_out", [NC, SL, PL], BF16)

    with tile.TileContext(nc) as tc:
        with tc.tile_pool(name="singles", bufs=1) as singles:
            # ---------------- phase 0: small loads ----------------
            m_off_t = singles.tile([128, NMT], I32)
            nc.sync.dma_start(out=m_off_t, in_=m_off[:, :])
            p_off_t = singles.tile([128, 2 * NPT], I32)
            nc.sync.dma_start(out=p_off_t, in_=p_off[:, :])
            ohe_t = singles.tile([128, NMT, E], BF16)
            nc.sync.dma_start(out=ohe_t, in_=ohe.rearrange("(t p) e -> p t e", p=128))
            ohm_t = singles.tile([128, NMT, E], BF16)
            nc.sync.dma_start(out=ohm_t, in_=ohm.rearrange("(t p) e -> p t e", p=128))
            ohh_t = singles.tile([128, 2, PL], BF16)
            nc.sync.dma_start(out=ohh_t, in_=ohh_k.rearrange("(t p) q -> p t q", p=128))
            oht_t = singles.tile([128, 2, PL], BF16)
            nc.sync.dma_start(out=oht_t, in_=oht_k.rearrange("(t p) q -> p t q", p=128))
            has0b = singles.tile([128, E], F32)
            nc.sync.dma_start(out=has0b, in_=has0r[:, :].to_broadcast([128, E]))
            bhs_t = singles.tile([128, PH // 128], F32)
            nc.sync.dma_start(out=bhs_t, in_=b_head[:, :])
            bts_t = singles.tile([128, PH // 128], F32)
            nc.sync.dma_start(out=bts_t, in_=b_tail[:, :])
            bbil_t = singles.tile([1, 1], F32)
            nc.sync.dma_start(out=bbil_t, in_=b_bil[:, :])
            ident = singles.tile([128, 128], BF16)
            make_identity(nc, ident[:, :])
            warm = singles.tile([1, 8], F32)
            nc.vector.memset(warm[:, :], 0.0)
            nc.scalar.activation(out=warm[:, :], in_=warm[:, :], func=AF.Tanh)
            ones_col = singles.tile([128, 1], BF16)
            nc.vector.memset(ones_col[:, :], 1.0)
            ones_row = singles.tile([1, 128], BF16)
            nc.vector.memset(ones_row[:, :], 1.0)

            entTe = singles.tile([128, H // 128, E], BF16)  # ent_emb^T [hcol-part, hc, e]
            RAW = singles.tile([128, NPT, SL], BF16)        # [p-row, pt, s]
            rawT = singles.tile([128, 2, NPT, 128], BF16)   # [s-part, sh, pt, p-row]
            paT = singles.tile([128, S // 128, PL], BF16)   # RAW^T for my pairs, all s
            ucb = singles.tile([128, H // 128, PL], BF16)   # contexts^T (unnormalized)
            ctxT = singles.tile([128, H // 128, PL], BF16)
            zsT = singles.tile([128, PH // 128, PL], BF16)
            zoT = singles.tile([128, PH // 128, PL], BF16)
            EWh = singles.tile([128, 2, PH], BF16)          # ent_emb @ W_head [e-part, et, PH]
            EWt = singles.tile([128, 2, PH], BF16)
            zrow = singles.tile([1, PL], F32)
            zrec = singles.tile([128, PL], BF16)
            lg_sb = singles.tile([1, PL], F32)

            # ---------------- phase 1: gathers + entity aggregation ----------------
            # All gather dest tiles share one pool scope: pools that close early
            # would let later pools reuse their SBUF, which inserts false WAR
            # waits into the in-order SWDGE stream and stalls the pair gathers.
            ev = singles.tile([128, NMT, H], BF16)  # exp(vals) [m-part, mt, hcol]
            with tc.tile_pool(name="p1", bufs=1) as p1, \
                 tc.tile_pool(name="ps_a", bufs=1, space="PSUM") as ps_a:
                entA_sb = p1.tile([128, 2, HEADS * SL], BF16)  # [e-part, et, (h s)]
                ag = []
                for mt in range(NMT):
                    # SWDGE casts f32->bf16 during the gather
                    g = p1.tile([128, HEADS * SL], BF16, tag=f"ag{mt}")
                    nc.gpsimd.indirect_dma_start(
                        out=g[:, :], out_offset=None, in_=att_k[:, :],
                        in_offset=bass.IndirectOffsetOnAxis(ap=m_off_t[:, mt:mt + 1], axis=0))
                    ag.append(g)
                for mt in range(NMT):
                    vg = p1.tile([128, H], F32, tag=f"vg{mt}")
                    nc.gpsimd.indirect_dma_start(
                        out=vg[:, :], out_offset=None, in_=seq[:, :],
                        in_offset=bass.IndirectOffsetOnAxis(ap=m_off_t[:, mt:mt + 1], axis=0))
                    nc.scalar.activation(out=ev[:, mt, :], in_=vg[:, :], func=AF.Exp)
                # heads in 2 groups of 8 (PSUM capacity); e in 2 half-tiles
                for hg in range(2):
                    for et in range(2):
                        pa = ps_a.tile([128, 8 * SL], F32, space="PSUM", tag="agg")
                        for mt in range(NMT):
                            for nch in range(4):  # 2048 = 4 x 512
                                nc.tensor.matmul(
                                    pa[:, nch * 512:(nch + 1) * 512],
                                    ohm_t[:, mt, et * 128:(et + 1) * 128],
                                    ag[mt][:, hg * 2048 + nch * 512: hg * 2048 + (nch + 1) * 512],
                                    start=(mt == 0), stop=(mt == NMT - 1))
                        nc.vector.tensor_copy(
                            out=entA_sb[:, et, hg * 2048:(hg + 1) * 2048],
                            in_=pa[:, :])
                        nc.sync.dma_start(
                            out=entA_dram.rearrange("(t p) w -> p t w", p=128)[
                                :, et, hg * 2048:(hg + 1) * 2048],
                            in_=entA_sb[:, et, hg * 2048:(hg + 1) * 2048])
                if debug:
                    nc.sync.dma_start(
                        out=dbg["entA"].rearrange("(t p) w -> p t w", p=128), in_=entA_sb)

            # ---------------- phase 1b-ii: logsumexp matmuls ----------------
            with tc.tile_pool(name="ps_s", bufs=2, space="PSUM") as ps_s:
                for hc in range(H // 128):
                    sp = ps_s.tile([128, E], F32, space="PSUM", tag="sums")
                    for mt in range(NMT):
                        nc.tensor.matmul(
                            sp[:, :], ev[:, mt, hc * 128:(hc + 1) * 128],
                            ohe_t[:, mt, :], start=(mt == 0), stop=(mt == NMT - 1))
                    nc.vector.tensor_tensor(out=sp[:, :], in0=sp[:, :], in1=has0b[:, :],
                                            op=OP.add)
                    nc.scalar.activation(out=entTe[:, hc, :], in_=sp[:, :], func=AF.Ln)
            if debug:
                nc.sync.dma_start(
                    out=dbg["ent_embT"].rearrange("(t p) e -> p t e", p=128), in_=entTe)

            # ---------------- phase 2: pair products ----------------
            with tc.tile_pool(name="pg", bufs=3) as pg, \
                 tc.tile_pool(name="prod", bufs=2) as prod, \
                 tc.tile_pool(name="ps_t", bufs=4, space="PSUM") as ps_t:
                for pt in range(NPT):
                    th = pg.tile([128, HEADS * SL], BF16, tag="th")
                    nc.gpsimd.indirect_dma_start(
                        out=th[:, :], out_offset=None, in_=entA_dram[:, :],
                        in_offset=bass.IndirectOffsetOnAxis(
                            ap=p_off_t[:, 2 * pt:2 * pt + 1], axis=0))
                    tt = pg.tile([128, HEADS * SL], BF16, tag="tt")
                    nc.gpsimd.indirect_dma_start(
                        out=tt[:, :], out_offset=None, in_=entA_dram[:, :],
                        in_offset=bass.IndirectOffsetOnAxis(
                            ap=p_off_t[:, 2 * pt + 1:2 * pt + 2], axis=0))
                    pr = prod.tile([128, HEADS * SL], BF16, tag="pr")
                    t1 = prod.tile([128, 8 * SL], BF16, tag="t1")
                    t2 = prod.tile([128, 4 * SL], BF16, tag="t2")
                    t3 = prod.tile([128, 2 * SL], BF16, tag="t3")
                    nc.vector.tensor_tensor(out=pr[:, :], in0=th[:, :], in1=tt[:, :],
                                            op=OP.mult)
                    nc.vector.tensor_tensor(out=t1[:, :], in0=pr[:, :8 * SL],
                                            in1=pr[:, 8 * SL:], op=OP.add)
                    nc.vector.tensor_tensor(out=t2[:, :], in0=t1[:, :4 * SL],
                                            in1=t1[:, 4 * SL:], op=OP.add)
                    nc.vector.tensor_tensor(out=t3[:, :], in0=t2[:, :2 * SL],
                                            in1=t2[:, 2 * SL:], op=OP.add)
                    nc.vector.tensor_tensor(out=RAW[:, pt, :], in0=t3[:, :SL],
                                            in1=t3[:, SL:], op=OP.add)
                    # transpose inline (PE is idle during products) and stage the
                    # AllToAll chunk as soon as its two pair-tiles are done.
                    for sh in range(2):
                        tp = ps_t.tile([128, 128], BF16, space="PSUM", tag="tp")
                        nc.tensor.transpose(
                            out=tp[:, :], in_=RAW[:, pt, sh * 128:(sh + 1) * 128],
                            identity=ident[:, :])
                        nc.vector.tensor_copy(out=rawT[:, sh, pt, :], in_=tp[:, :])
                    if pt % 2 == 1:
                        c = pt // 2
                        nc.sync.dma_start(
                            out=a2a_in[c].rearrange("(sh sp) (pl pr) -> sp sh pl pr", sh=2, pl=2),
                            in_=rawT[:, :, 2 * c:2 * c + 2, :])
            # weights + seqext (bf16 SWDGE cast loads). Emitted AFTER the pair
            # gathers so the Q7 descriptor stream doesn't delay them. whb/wtb go
            # first so the EW matmuls can fill the PE while the AllToAll runs.
            with tc.tile_pool(name="wpool", bufs=1) as wpool:
                whb = wpool.tile([128, H // 128, PH], BF16)
                nc.gpsimd.dma_start(out=whb, in_=w_head.rearrange("(t p) n -> p t n", p=128))
                wtb = wpool.tile([128, H // 128, PH], BF16)
                nc.gpsimd.dma_start(out=wtb, in_=w_tail.rearrange("(t p) n -> p t n", p=128))

                # ---------------- phase 4b: EW = ent_emb @ W (fills A2A window) -----
                with tc.tile_pool(name="ps_e", bufs=2, space="PSUM") as ps_e:
                    for (wsb, dst) in ((whb, EWh), (wtb, EWt)):
                        for et in range(2):
                            ep = ps_e.tile([128, PH], F32, space="PSUM", tag="ew")
                            for kt in range(H // 128):
                                for nch in range(2):
                                    nc.tensor.matmul(
                                        ep[:, nch * 512:(nch + 1) * 512],
                                        entTe[:, kt, et * 128:(et + 1) * 128],
                                        wsb[:, kt, nch * 512:(nch + 1) * 512],
                                        start=(kt == 0), stop=(kt == H // 128 - 1))
                            nc.scalar.copy(out=dst[:, et, :], in_=ep[:, :])

                wcb = wpool.tile([128, H // 128, PH], BF16)
                nc.gpsimd.dma_start(out=wcb, in_=w_ctx.rearrange("(t p) n -> p t n", p=128))
                seqx = wpool.tile([128, S // 128, H + 1], BF16)
                nc.gpsimd.dma_start(
                    out=seqx[:, :, 0:H], in_=seq.rearrange("(t p) h -> p t h", p=128))
                nc.vector.memset(seqx[:, :, H:H + 1], 1.0)
                wbb = wpool.tile([128, PH // 128, PH], BF16)
                nc.gpsimd.dma_start(out=wbb, in_=w_bil.rearrange("(t p) n -> p t n", p=128))

                # ---------------- phase 3: AllToAll (staged incrementally above) ----
                nc.gpsimd.collective_compute(
                    "AllToAll", OP.bypass, replica_groups=[list(range(NC))],
                    ins=[a2a_in[:, :, :]], outs=[a2a_out[:, :, :]])
                nc.sync.dma_start(
                    out=paT, in_=a2a_out.rearrange("j (sh sp) q -> sp (j sh) q", sh=2))

                # ---------------- phase 4: uc^T = [seq|1]^T @ pa ----------------
                with tc.tile_pool(name="ps_u", bufs=2, space="PSUM") as ps_u:
                    zp = ps_u.tile([1, PL], F32, space="PSUM", tag="zr")
                    for t in range(S // 128):
                        nc.tensor.matmul(
                            zp[:, :], seqx[:, t, H:H + 1], paT[:, t, :],
                            start=(t == 0), stop=(t == S // 128 - 1))
                    nc.vector.tensor_copy(out=zrow[:, :], in_=zp[:, :])
                    for mc in range(H // 128):
                        up = ps_u.tile([128, PL], F32, space="PSUM", tag="uc")
                        for t in range(S // 128):
                            nc.tensor.matmul(
                                up[:, :], seqx[:, t, mc * 128:(mc + 1) * 128],
                                paT[:, t, :], start=(t == 0), stop=(t == S // 128 - 1))
                        nc.vector.tensor_copy(out=ucb[:, mc, :], in_=up[:, :])
                if debug:
                    nc.sync.dma_start(
                        out=dbg["ctxuT"].rearrange("(t p) q -> p t q", p=128), in_=ucb)
                    nc.sync.dma_start(out=dbg["zrow"][:, :], in_=zrow)

                # recip(Z + 1e-6) -> broadcast to 128 partitions via K=1 matmul
                nc.vector.tensor_scalar_add(out=zrow[:, :], in0=zrow[:, :], scalar1=1e-6)
                nc.vector.reciprocal(out=zrow[:, :], in_=zrow[:, :])
                zrec_b = singles.tile([1, PL], BF16)
                nc.vector.tensor_copy(out=zrec_b, in_=zrow[:, :])
                with tc.tile_pool(name="ps_r", bufs=1, space="PSUM") as ps_r:
                    zrp = ps_r.tile([128, PL], F32, space="PSUM", tag="zrp")
                    nc.tensor.matmul(zrp[:, :], ones_row[:, :],
                                     zrec_b[:, :], start=True, stop=True)
                    nc.vector.tensor_copy(out=zrec, in_=zrp[:, :])
                for mc in range(H // 128):
                    nc.vector.tensor_tensor(out=ctxT[:, mc, :], in0=ucb[:, mc, :],
                                            in1=zrec[:, :], op=OP.mult)

                # ---------------- phase 5: z_s, z_o (ctx matmul shared) ----------------
                with tc.tile_pool(name="ps_z", bufs=2, space="PSUM") as ps_z, \
                     tc.tile_pool(name="zscr", bufs=2) as zscr:
                    for jt in range(PH // 128):
                        cps = ps_z.tile([128, PL], F32, space="PSUM", tag="cp")
                        for kt in range(H // 128):
                            nc.tensor.matmul(
                                cps[:, :], wcb[:, kt, jt * 128:(jt + 1) * 128],
                                ctxT[:, kt, :], start=(kt == 0), stop=(kt == H // 128 - 1))
                        cpsb = zscr.tile([128, PL], BF16, tag="cpsb")
                        nc.scalar.copy(out=cpsb[:, :], in_=cps[:, :])
                        for (ew, oh, bias, dst, tg) in ((EWh, ohh_t, bhs_t, zsT, "zs"),
                                                        (EWt, oht_t, bts_t, zoT, "zo")):
                            zps = ps_z.tile([128, PL], F32, space="PSUM", tag=tg)
                            for et in range(2):
                                nc.tensor.matmul(
                                    zps[:, :], ew[:, et, jt * 128:(jt + 1) * 128],
                                    oh[:, et, :], start=(et == 0), stop=(et == 1))
                            nc.vector.tensor_tensor(out=zps[:, :], in0=zps[:, :],
                                                    in1=cpsb[:, :], op=OP.add)
                            nc.scalar.activation(out=dst[:, jt, :], in_=zps[:, :],
                                                 func=AF.Tanh, bias=bias[:, jt:jt + 1])
                # ---------------- phase 6: bilinear ----------------
                with tc.tile_pool(name="ps_b", bufs=3, space="PSUM") as ps_b, \
                     tc.tile_pool(name="bprod", bufs=2) as bprod:
                    lg = ps_b.tile([1, PL], F32, space="PSUM", tag="lg")
                    for jt in range(PH // 128):
                        ups = ps_b.tile([128, PL], F32, space="PSUM", tag="u")
                        for it in range(PH // 128):
                            nc.tensor.matmul(
                                ups[:, :], wbb[:, it, jt * 128:(jt + 1) * 128],
                                zsT[:, it, :], start=(it == 0), stop=(it == PH // 128 - 1))
                        pb = bprod.tile([128, PL], BF16, tag="pb")
                        nc.vector.tensor_tensor(out=pb[:, :], in0=ups[:, :],
                                                in1=zoT[:, jt, :], op=OP.mult)
                        nc.tensor.matmul(
                            lg[:, :], ones_col[:, :], pb[:, :],
                            start=(jt == 0), stop=(jt == PH // 128 - 1))
                    nc.vector.tensor_scalar_add(out=lg_sb[:, :], in0=lg[:, :],
                                                scalar1=bbil_t[:, 0:1])
                nc.sync.dma_start(out=out[:, :], in_=lg_sb)

    nc.finalize()
    return nc


def _get_nc(debug=False):
    key = ("nc", debug)
    if key not in _CACHE:
        _CACHE[key] = _build(debug)
    return _CACHE[key]


def _prep_in_maps(inputs):
    import ml_dtypes
    bf16 = ml_dtypes.bfloat16

    att = np.asarray(inputs["attention"], np.float32)          # [16, 2048, 2048]
    seq = np.ascontiguousarray(np.asarray(inputs["sequence_output"], np.float32))
    mention_idx = np.asarray(inputs["mention_idx"], np.int32)  # [1024]
    entity_ids = np.asarray(inputs["entity_ids"], np.int32)    # [1024]
    pair_h = np.asarray(inputs["pair_h"], np.int32)            # [2048]
    pair_t = np.asarray(inputs["pair_t"], np.int32)

    counts = np.bincount(entity_ids, minlength=E).astype(np.float32)
    inv_cnt = 1.0 / np.maximum(counts, 1.0)

    ohe = np.zeros((NM, E), np.float32)
    ohe[np.arange(NM), entity_ids] = 1.0
    ohm = np.zeros((NM, E), np.float32)
    ohm[np.arange(NM), entity_ids] = inv_cnt[entity_ids]
    has0r = (counts == 0).astype(np.float32)[None, :]

    m_off = mention_idx.reshape(NMT, 128).T.copy()             # [128, 8]

    order = np.argsort(pair_h, kind="stable")
    sph = pair_h[order]
    spt = pair_t[order]
    p_off = np.zeros((128, 2 * NPT), np.int32)
    for pt in range(NPT):
        seg = slice(pt * 128, (pt + 1) * 128)
        p_off[:, 2 * pt] = sph[seg]
        p_off[:, 2 * pt + 1] = spt[seg]

    shared = {
        "seq": seq,
        "m_off": m_off,
        "p_off": p_off,
        "ohe": ohe.astype(bf16),
        "ohm": ohm.astype(bf16),
        "has0r": has0r,
        "w_head": np.asarray(inputs["W_head"], np.float32),
        "w_tail": np.asarray(inputs["W_tail"], np.float32),
        "w_ctx": np.asarray(inputs["W_ctx"], np.float32),
        "w_bil": np.asarray(inputs["W_bil"], np.float32),
        "b_head": np.asarray(inputs["b_head"], np.float32).reshape(PH // 128, 128).T.copy(),
        "b_tail": np.asarray(inputs["b_tail"], np.float32).reshape(PH // 128, 128).T.copy(),
        "b_bil": np.asarray(inputs["b_bil"], np.float32).reshape(1, 1),
    }

    in_maps = []
    for k in range(NC):
        sk = k * SL
        att_kk = np.ascontiguousarray(
            att[:, :, sk:sk + SL].transpose(1, 0, 2)).reshape(S, HEADS * SL)
        ohh_kk = np.zeros((E, PL), np.float32)
        ohh_kk[sph[k * PL:(k + 1) * PL], np.arange(PL)] = 1.0
        oht_kk = np.zeros((E, PL), np.float32)
        oht_kk[spt[k * PL:(k + 1) * PL], np.arange(PL)] = 1.0
        m = dict(shared)
        m["att_k"] = att_kk
        m["ohh_k"] = ohh_kk.astype(bf16)
        m["oht_k"] = oht_kk.astype(bf16)
        in_maps.append(m)
    return in_maps


def _run(inputs, trace=False, debug=False):
    _ensure_axon_profile_hook()
    from concourse.bass_utils import run_bass_kernel_spmd
    nc = _get_nc(debug)
    in_maps = _prep_in_maps(inputs)
    res = run_bass_kernel_spmd(nc, in_maps, list(range(NC)), trace=trace)
    sorted_logits = np.concatenate([np.asarray(res.results[k]["out"][0], np.float32)
                                    for k in range(NC)])
    order = np.argsort(np.asarray(inputs["pair_h"], np.int32), kind="stable")
    logits = np.empty(P, np.float32)
    logits[order] = sorted_logits
    return logits, res


def kernel(**inputs) -> np.ndarray:
    logits, _ = _run(inputs, trace=False)
    return logits


def kernel_traced(**inputs):
    logits, res = _run(inputs, trace=True)
    return logits, res


def kernel_debug(**inputs):
    logits, res = _run(inputs, trace=False, debug=True)
    return logits, res



# revision 6
# speedup vs baseline: 1.3162x; 1.3162x over previous
"""Trainium2 Bass kernel for nn_CandidateFilterModel (segment_reduce).

Strategy (8 cores, S-column sharding for the heavy phases, pair sharding for the tail):
  - Core k owns sequence-column slice s_k = [256k, 256k+256).
  - Phase 1: entity aggregation.
      ent_emb^T = log(OH_emb-matmul of exp(seq[mention_idx]))   (replicated, bf16)
      ent_att (local s-slice) = OH_mean-matmul of gathered attention rows (fp8)
      One-hot slabs that are all-zero (entity_ids is sorted, so each mention
      tile only spans ~32 entities) are skipped entirely.
  - Phase 2: pair products. For all 2048 pairs: gather ent_att rows of head/tail
      entity (4KB fp8 rows, indirect DMA), multiply (fp8 in, bf16 out), one DVE
      add folds 16 heads -> 8, then PE transpose-ACCUMULATE matmuls (x identity)
      fold the remaining 8 head-blocks while transposing -> raw^T in PSUM.
  - Phase 3: TWO AllToAlls (even pair-tiles = first 128 pairs of each dest
      core, then odd) redistribute raw^T so core k holds raw^T[:, P_k].
  - Phases 4-6 (per pair-half): contexts via seq^T-matmul, normalize, z_s/z_o
      via (ent_emb @ W)-then-gather one-hot matmuls + W_ctx matmuls + tanh,
      bilinear via W_bil matmuls + elementwise + ones-reduction matmul.
Host pre-casts: attention fp8 e4m3 (quantization error largely cancels in the
pair_att normalization), seq/weights bf16. DMA queues: gpsimd = indirect
gathers + collectives, sync = small loads/staging/paT, scalar = weight loads.
PSUM->SBUF copies in the tail ride the scalar engine to keep DVE free.
"""
import sys
import types
import numpy as np

S, H, HEADS = 2048, 1024, 16
E, NM, P = 256, 1024, 2048
PH = 1024
NC = 8
SL = S // NC          # 256 s-columns per core
PL = P // NC          # 256 pairs per core
NMT = NM // 128       # 8 mention tiles
NPT = P // 128        # 16 pair tiles
HS = HEADS * SL       # 4096 = width of per-core ent_att rows

_CACHE = {}


def _ensure_axon_profile_hook():
    """bass_utils' trace path imports antenv.axon_hooks, absent in this image."""
    if 'antenv.axon_hooks' in sys.modules:
        return
    try:
        import antenv.axon_hooks  # noqa: F401
        return
    except ImportError:
        pass
    mod = types.ModuleType('antenv.axon_hooks')
    holder = [None]
    mod.set_axon_ntff_profile_hook = lambda h: holder.__setitem__(0, h)
    mod.get_axon_ntff_profile_hook = lambda: holder[0]
    sys.modules['antenv.axon_hooks'] = mod
    try:
        from trn_agent_boot.trn_boot import _ntff_profile_via_ctypes
        hook = _ntff_profile_via_ctypes('/opt/axon/libaxon_pjrt.so')
        if hook is not None:
            mod.set_axon_ntff_profile_hook(hook)
    except Exception:
        pass


def _build(mt_ets, debug=False):
    """mt_ets: per mention-tile, tuple of entity-128-halves it touches."""
    import concourse.bass as bass
    import concourse.bacc as bacc
    import concourse.tile as tile
    from concourse import mybir
    from concourse.masks import make_identity

    F32 = mybir.dt.float32
    BF16 = mybir.dt.bfloat16
    F8 = mybir.dt.float8e4
    I32 = mybir.dt.int32
    AF = mybir.ActivationFunctionType
    OP = mybir.AluOpType

    nc = bacc.Bacc(num_devices=NC)

    # ---------------- inputs ----------------
    att_k = nc.declare_dram_parameter("att_k", [S, HS], F8, isOutput=False)
    seqb = nc.declare_dram_parameter("seqb", [S, H], BF16, isOutput=False)
    m_off = nc.declare_dram_parameter("m_off", [128, NMT], I32, isOutput=False)
    p_off = nc.declare_dram_parameter("p_off", [128, 2 * NPT], I32, isOutput=False)
    ohe = nc.declare_dram_parameter("ohe", [NM, E], BF16, isOutput=False)
    ohm = nc.declare_dram_parameter("ohm", [NM, E], F8, isOutput=False)
    has0r = nc.declare_dram_parameter("has0r", [1, E], F32, isOutput=False)
    ohh_k = nc.declare_dram_parameter("ohh_k", [E, PL], BF16, isOutput=False)
    oht_k = nc.declare_dram_parameter("oht_k", [E, PL], BF16, isOutput=False)
    w_head = nc.declare_dram_parameter("w_head", [H, PH], BF16, isOutput=False)
    w_tail = nc.declare_dram_parameter("w_tail", [H, PH], BF16, isOutput=False)
    w_ctx = nc.declare_dram_parameter("w_ctx", [H, PH], BF16, isOutput=False)
    w_bil = nc.declare_dram_parameter("w_bil", [PH, PH], BF16, isOutput=False)
    b_head = nc.declare_dram_parameter("b_head", [128, PH // 128], F32, isOutput=False)
    b_tail = nc.declare_dram_parameter("b_tail", [128, PH // 128], F32, isOutput=False)
    b_bil = nc.declare_dram_parameter("b_bil", [1, 1], F32, isOutput=False)
    out = nc.declare_dram_parameter("out", [1, PL], F32, isOutput=True)

    dbg = {}
    if debug:
        dbg["ent_embT"] = nc.declare_dram_parameter("d_ent_embT", [H, E], BF16, isOutput=True)
        dbg["entA"] = nc.declare_dram_parameter("d_entA", [E, HS], BF16, isOutput=True)
        dbg["rawT"] = nc.declare_dram_parameter("d_rawT", [128, 2 * NPT * 128], BF16, isOutput=True)
        dbg["ctxuT"] = nc.declare_dram_parameter("d_ctxuT", [H, PL], BF16, isOutput=True)
        dbg["zrow"] = nc.declare_dram_parameter("d_zrow", [1, PL], F32, isOutput=True)
        dbg["zsT"] = nc.declare_dram_parameter("d_zsT", [PH, PL], BF16, isOutput=True)

    # internal DRAM
    entA_dram = nc.dram_tensor("entA_dram", [E, HS], F8)
    a2a_in = [nc.dram_tensor(f"a2a{h}_in", [NC, SL, 128], BF16) for h in range(2)]
    a2a_out = [nc.dram_tensor(f"a2a{h}_out", [NC, SL, 128], BF16) for h in range(2)]

    et_mts = {0: [mt for mt in range(NMT) if 0 in mt_ets[mt]],
              1: [mt for mt in range(NMT) if 1 in mt_ets[mt]]}

    with tile.TileContext(nc) as tc:
        with tc.tile_pool(name="singles", bufs=1) as singles, \
             tc.tile_pool(name="wpool", bufs=1) as wpool:
            # ---------------- phase 0: small loads (sync queue) ----------------
            m_off_t = singles.tile([128, NMT], I32)
            nc.sync.dma_start(out=m_off_t, in_=m_off[:, :])
            p_off_t = singles.tile([128, 2 * NPT], I32)
            nc.sync.dma_start(out=p_off_t, in_=p_off[:, :])
            ohe_t = singles.tile([128, NMT, E], BF16)
            nc.sync.dma_start(out=ohe_t, in_=ohe.rearrange("(t p) e -> p t e", p=128))
            ohm_t = singles.tile([128, NMT, E], F8)
            nc.sync.dma_start(out=ohm_t, in_=ohm.rearrange("(t p) e -> p t e", p=128))
            ohh_t = singles.tile([128, 2, PL], BF16)
            nc.sync.dma_start(out=ohh_t, in_=ohh_k.rearrange("(t p) q -> p t q", p=128))
            oht_t = singles.tile([128, 2, PL], BF16)
            nc.sync.dma_start(out=oht_t, in_=oht_k.rearrange("(t p) q -> p t q", p=128))
            has0b = singles.tile([128, E], F32)
            nc.sync.dma_start(out=has0b, in_=has0r[:, :].to_broadcast([128, E]))
            bhs_t = singles.tile([128, PH // 128], F32)
            nc.sync.dma_start(out=bhs_t, in_=b_head[:, :])
            bts_t = singles.tile([128, PH // 128], F32)
            nc.sync.dma_start(out=bts_t, in_=b_tail[:, :])
            bbil_t = singles.tile([1, 1], F32)
            nc.sync.dma_start(out=bbil_t, in_=b_bil[:, :])
            ident = singles.tile([128, 128], BF16)
            make_identity(nc, ident[:, :])
            # warm activation tables; Exp last = first real user
            warm = singles.tile([1, 8], F32)
            nc.vector.memset(warm[:, :], 0.0)
            nc.scalar.activation(out=warm[:, :], in_=warm[:, :], func=AF.Tanh)
            nc.scalar.activation(out=warm[:, :], in_=warm[:, :], func=AF.Ln)
            nc.scalar.activation(out=warm[:, :], in_=warm[:, :], func=AF.Exp)
            ones_col = singles.tile([128, 1], BF16)
            nc.vector.memset(ones_col[:, :], 1.0)
            ones_row = singles.tile([1, 128], BF16)
            nc.vector.memset(ones_row[:, :], 1.0)

            entTe = singles.tile([128, H // 128, E], BF16)  # ent_emb^T [hcol-part, hc, e]
            rawT = singles.tile([128, 2, NPT, 128], BF16)   # [s-part, sh, pt, p-row]
            paT = singles.tile([128, S // 128, PL], BF16)   # raw^T for my pairs, all s
            ucb = singles.tile([128, H // 128, PL], BF16)   # contexts^T (unnormalized)
            ctxT = singles.tile([128, H // 128, PL], BF16)
            zsT = singles.tile([128, PH // 128, PL], BF16)
            zoT = singles.tile([128, PH // 128, PL], BF16)
            EWh = singles.tile([128, 2, PH], BF16)          # ent_emb @ W_head [e-part, et, PH]
            EWt = singles.tile([128, 2, PH], BF16)
            zrow = singles.tile([1, PL], F32)
            zrec = singles.tile([128, PL], BF16)
            zrec_b = singles.tile([1, PL], BF16)
            lg_sb = singles.tile([1, PL], F32)

            # weight loads (scalar HWDGE queue — parallel to gpsimd gathers).
            # whb/wtb first: needed by the EW matmuls that fill PE during ph2.
            whb = wpool.tile([128, H // 128, PH], BF16)
            nc.scalar.dma_start(out=whb, in_=w_head.rearrange("(t p) n -> p t n", p=128))
            wtb = wpool.tile([128, H // 128, PH], BF16)
            nc.scalar.dma_start(out=wtb, in_=w_tail.rearrange("(t p) n -> p t n", p=128))
            wcb = wpool.tile([128, H // 128, PH], BF16)
            wbb = wpool.tile([128, PH // 128, PH], BF16)
            seqx = wpool.tile([128, S // 128, H], BF16)

            # ---------------- phase 1: gathers + entity aggregation ----------------
            with tc.tile_pool(name="p1", bufs=1) as p1:
                ag = []
                for mt in range(NMT):
                    g = p1.tile([128, HS], F8, tag=f"ag{mt}")
                    nc.gpsimd.indirect_dma_start(
                        out=g[:, :], out_offset=None, in_=att_k[:, :],
                        in_offset=bass.IndirectOffsetOnAxis(ap=m_off_t[:, mt:mt + 1], axis=0))
                    ag.append(g)
                ev = []
                for mt in range(NMT):
                    vg = p1.tile([128, H], BF16, tag=f"vg{mt}")
                    nc.gpsimd.indirect_dma_start(
                        out=vg[:, :], out_offset=None, in_=seqb[:, :],
                        in_offset=bass.IndirectOffsetOnAxis(ap=m_off_t[:, mt:mt + 1], axis=0))
                    nc.scalar.activation(out=vg[:, :], in_=vg[:, :], func=AF.Exp)
                    ev.append(vg)

                # remaining big loads (scalar queue; stream during ph1/ph2)
                nc.scalar.dma_start(out=wcb, in_=w_ctx.rearrange("(t p) n -> p t n", p=128))
                nc.scalar.dma_start(out=seqx, in_=seqb.rearrange("(t p) h -> p t h", p=128))
                nc.scalar.dma_start(out=wbb, in_=w_bil.rearrange("(t p) n -> p t n", p=128))

                # heads in 2 groups of 8 (PSUM capacity); both entity halves
                # accumulate concurrently; all-zero (mt, et) slabs are skipped.
                with tc.tile_pool(name="ps_a", bufs=1, space="PSUM") as ps_a:
                    for hg in range(2):
                        pas0 = ps_a.tile([128, 8 * SL], F32, space="PSUM", tag="agg0")
                        pas1 = ps_a.tile([128, 8 * SL], F32, space="PSUM", tag="agg1")
                        pas = {0: pas0, 1: pas1}
                        for et in range(2):
                            if not et_mts[et]:
                                nc.vector.memset(pas[et][:, :], 0.0)
                        for mt in range(NMT):
                            for et in mt_ets[mt]:
                                for nch in range(4):  # 2048 = 4 x 512
                                    nc.tensor.matmul(
                                        pas[et][:, nch * 512:(nch + 1) * 512],
                                        ohm_t[:, mt, et * 128:(et + 1) * 128],
                                        ag[mt][:, hg * 2048 + nch * 512:
                                               hg * 2048 + (nch + 1) * 512],
                                        start=(mt == et_mts[et][0]),
                                        stop=(mt == et_mts[et][-1]))
                        for et in range(2):
                            eA = p1.tile([128, 8 * SL], F8, tag="entA_sb")
                            nc.vector.tensor_copy(out=eA[:, :], in_=pas[et][:, :])
                            nc.sync.dma_start(
                                out=entA_dram.rearrange("(t p) w -> p t w", p=128)[
                                    :, et, hg * 2048:(hg + 1) * 2048],
                                in_=eA[:, :])
                            if debug:
                                eAb = p1.tile([128, 8 * SL], BF16, tag="entA_dbg")
                                nc.vector.tensor_copy(out=eAb[:, :], in_=pas[et][:, :])
                                nc.sync.dma_start(
                                    out=dbg["entA"].rearrange("(t p) w -> p t w", p=128)[
                                        :, et, hg * 2048:(hg + 1) * 2048],
                                    in_=eAb[:, :])

                # ---------------- phase 1b: logsumexp matmuls ----------------
                with tc.tile_pool(name="ps_s", bufs=2, space="PSUM") as ps_s:
                    for hc in range(H // 128):
                        sp = ps_s.tile([128, E], F32, space="PSUM", tag="sums")
                        for et in range(2):
                            mts = et_mts[et]
                            if not mts:
                                nc.vector.memset(sp[:, et * 128:(et + 1) * 128], 0.0)
                                continue
                            for mt in mts:
                                nc.tensor.matmul(
                                    sp[:, et * 128:(et + 1) * 128],
                                    ev[mt][:, hc * 128:(hc + 1) * 128],
                                    ohe_t[:, mt, et * 128:(et + 1) * 128],
                                    start=(mt == mts[0]), stop=(mt == mts[-1]))
                        nc.vector.tensor_tensor(out=sp[:, :], in0=sp[:, :],
                                                in1=has0b[:, :], op=OP.add)
                        nc.scalar.activation(out=entTe[:, hc, :], in_=sp[:, :], func=AF.Ln)
            if debug:
                nc.sync.dma_start(
                    out=dbg["ent_embT"].rearrange("(t p) e -> p t e", p=128), in_=entTe)

            # ---------------- phase 4b: EW = ent_emb @ W (PE, early) ----------------
            with tc.tile_pool(name="ps_e", bufs=2, space="PSUM") as ps_e:
                for (wsb, dstw) in ((whb, EWh), (wtb, EWt)):
                    for et in range(2):
                        ep = ps_e.tile([128, PH], F32, space="PSUM", tag="ew")
                        for kt in range(H // 128):
                            for nch in range(2):
                                nc.tensor.matmul(
                                    ep[:, nch * 512:(nch + 1) * 512],
                                    entTe[:, kt, et * 128:(et + 1) * 128],
                                    wsb[:, kt, nch * 512:(nch + 1) * 512],
                                    start=(kt == 0), stop=(kt == H // 128 - 1))
                        nc.scalar.copy(out=dstw[:, et, :], in_=ep[:, :])

            # ---------------- phase 2: pair products ----------------
            # evens (tiles 0,2,..,14 = first 128 pairs of each dest core) first
            # so AllToAll #A can fire while the odds still stream.
            def pair_tile(pt, pg, prod, ps_r):
                th = pg.tile([128, HS], F8, tag="th")
                nc.gpsimd.indirect_dma_start(
                    out=th[:, :], out_offset=None, in_=entA_dram[:, :],
                    in_offset=bass.IndirectOffsetOnAxis(
                        ap=p_off_t[:, 2 * pt:2 * pt + 1], axis=0))
                tt = pg.tile([128, HS], F8, tag="tt")
                nc.gpsimd.indirect_dma_start(
                    out=tt[:, :], out_offset=None, in_=entA_dram[:, :],
                    in_offset=bass.IndirectOffsetOnAxis(
                        ap=p_off_t[:, 2 * pt + 1:2 * pt + 2], axis=0))
                pr = prod.tile([128, HS], BF16, tag="pr")
                nc.vector.tensor_tensor(out=pr[:, :], in0=th[:, :], in1=tt[:, :],
                                        op=OP.mult)
                # fold 16 heads -> 8 on DVE; remaining 8 fold inside the
                # transpose-accumulate matmuls (x identity) on PE.
                nc.vector.tensor_tensor(out=pr[:, :8 * SL], in0=pr[:, :8 * SL],
                                        in1=pr[:, 8 * SL:], op=OP.add)
                rp = ps_r.tile([128, 2, 128], F32, space="PSUM", tag="rp")
                for sh in range(2):
                    for hb in range(8):
                        nc.tensor.matmul(
                            rp[:, sh, :],
                            pr[:, hb * SL + sh * 128: hb * SL + sh * 128 + 128],
                            ident[:, :], start=(hb == 0), stop=(hb == 7))
                    nc.scalar.copy(out=rawT[:, sh, pt, :], in_=rp[:, sh, :])
                c, odd = pt // 2, pt % 2
                nc.sync.dma_start(
                    out=a2a_in[odd][c].rearrange("(sh sp) p -> sp sh p", sh=2),
                    in_=rawT[:, :, pt, :])

            with tc.tile_pool(name="pg", bufs=3) as pg, \
                 tc.tile_pool(name="prod", bufs=2) as prod, \
                 tc.tile_pool(name="ps_r", bufs=3, space="PSUM") as ps_r:
                for c in range(NC):
                    pair_tile(2 * c, pg, prod, ps_r)
                # a2a #A fires as soon as the evens are staged; odds still run.
                nc.gpsimd.collective_compute(
                    "AllToAll", OP.bypass, replica_groups=[list(range(NC))],
                    ins=[a2a_in[0][:, :, :]], outs=[a2a_out[0][:, :, :]])
                nc.sync.dma_start(
                    out=paT[:, :, 0:128],
                    in_=a2a_out[0].rearrange("j (sh sp) q -> sp (j sh) q", sh=2))
                for c in range(NC):
                    pair_tile(2 * c + 1, pg, prod, ps_r)
                nc.gpsimd.collective_compute(
                    "AllToAll", OP.bypass, replica_groups=[list(range(NC))],
                    ins=[a2a_in[1][:, :, :]], outs=[a2a_out[1][:, :, :]])
                nc.sync.dma_start(
                    out=paT[:, :, 128:256],
                    in_=a2a_out[1].rearrange("j (sh sp) q -> sp (j sh) q", sh=2))
                if debug:
                    nc.sync.dma_start(
                        out=dbg["rawT"][:, :],
                        in_=rawT.rearrange("p a b c -> p (a b c)"))

            # ---------------- phases 4..6: pair-half tails ----------------
            with tc.tile_pool(name="ps_tail", bufs=2, space="PSUM") as ps_tail, \
                 tc.tile_pool(name="zscr", bufs=2) as zscr:
                ps_u = ps_z = ps_b = ps_tail
                for hf in range(2):
                    q0, q1 = hf * 128, hf * 128 + 128

                    # ---- contexts (unnormalized) + Z ----
                    zp = ps_u.tile([1, 128], F32, space="PSUM", tag="zr")
                    for t in range(S // 128):
                        nc.tensor.matmul(
                            zp[:, :], ones_col[:, :], paT[:, t, q0:q1],
                            start=(t == 0), stop=(t == S // 128 - 1))
                    nc.scalar.copy(out=zrow[:, q0:q1], in_=zp[:, :])
                    for mc in range(H // 128):
                        up = ps_u.tile([128, 128], F32, space="PSUM", tag="work")
                        for t in range(S // 128):
                            nc.tensor.matmul(
                                up[:, :], seqx[:, t, mc * 128:(mc + 1) * 128],
                                paT[:, t, q0:q1], start=(t == 0),
                                stop=(t == S // 128 - 1))
                        nc.scalar.copy(out=ucb[:, mc, q0:q1], in_=up[:, :])

                    # recip(Z + 1e-6) -> broadcast to 128 partitions via K=1 matmul
                    nc.vector.tensor_scalar_add(out=zrow[:, q0:q1], in0=zrow[:, q0:q1],
                                                scalar1=1e-6)
                    nc.vector.reciprocal(out=zrow[:, q0:q1], in_=zrow[:, q0:q1])
                    nc.vector.tensor_copy(out=zrec_b[:, q0:q1], in_=zrow[:, q0:q1])
                    zrp = ps_u.tile([128, 128], F32, space="PSUM", tag="work")
                    nc.tensor.matmul(zrp[:, :], ones_row[:, :], zrec_b[:, q0:q1],
                                     start=True, stop=True)
                    nc.scalar.copy(out=zrec[:, q0:q1], in_=zrp[:, :])
                    for mc in range(H // 128):
                        nc.vector.tensor_tensor(out=ctxT[:, mc, q0:q1],
                                                in0=ucb[:, mc, q0:q1],
                                                in1=zrec[:, q0:q1], op=OP.mult)

                    # ---- z_s, z_o (ctx matmul shared) ----
                    for jt in range(PH // 128):
                        cps = ps_z.tile([128, 128], F32, space="PSUM", tag="work")
                        for kt in range(H // 128):
                            nc.tensor.matmul(
                                cps[:, :], wcb[:, kt, jt * 128:(jt + 1) * 128],
                                ctxT[:, kt, q0:q1], start=(kt == 0),
                                stop=(kt == H // 128 - 1))
                        cpsb = zscr.tile([128, 128], BF16, tag="cpsb")
                        nc.scalar.copy(out=cpsb[:, :], in_=cps[:, :])
                        for (ew, oh, bias, dstz, tg) in ((EWh, ohh_t, bhs_t, zsT, "zs"),
                                                         (EWt, oht_t, bts_t, zoT, "zo")):
                            zps = ps_z.tile([128, 128], F32, space="PSUM", tag="zps")
                            for et in range(2):
                                nc.tensor.matmul(
                                    zps[:, :], ew[:, et, jt * 128:(jt + 1) * 128],
                                    oh[:, et, q0:q1], start=(et == 0), stop=(et == 1))
                            nc.vector.tensor_tensor(out=zps[:, :], in0=zps[:, :],
                                                    in1=cpsb[:, :], op=OP.add)
                            nc.scalar.activation(out=dstz[:, jt, q0:q1], in_=zps[:, :],
                                                 func=AF.Tanh, bias=bias[:, jt:jt + 1])

                    # ---- bilinear ----
                    lg = ps_b.tile([1, 128], F32, space="PSUM", tag="lg")
                    for jt in range(PH // 128):
                        ups = ps_b.tile([128, 128], F32, space="PSUM", tag="work")
                        for it in range(PH // 128):
                            nc.tensor.matmul(
                                ups[:, :], wbb[:, it, jt * 128:(jt + 1) * 128],
                                zsT[:, it, q0:q1], start=(it == 0),
                                stop=(it == PH // 128 - 1))
                        pb = zscr.tile([128, 128], BF16, tag="pb")
                        nc.vector.tensor_tensor(out=pb[:, :], in0=ups[:, :],
                                                in1=zoT[:, jt, q0:q1], op=OP.mult)
                        nc.tensor.matmul(
                            lg[:, :], ones_col[:, :], pb[:, :],
                            start=(jt == 0), stop=(jt == PH // 128 - 1))
                    nc.vector.tensor_scalar_add(out=lg_sb[:, q0:q1], in0=lg[:, :],
                                                scalar1=bbil_t[:, 0:1])
                if debug:
                    nc.sync.dma_start(
                        out=dbg["ctxuT"].rearrange("(t p) q -> p t q", p=128), in_=ucb)
                    nc.sync.dma_start(out=dbg["zrow"][:, :], in_=zrow)
                    nc.sync.dma_start(
                        out=dbg["zsT"].rearrange("(t p) q -> p t q", p=128), in_=zsT)
                nc.sync.dma_start(out=out[:, :], in_=lg_sb)

    nc.finalize()
    return nc


def _get_nc(mt_ets, debug=False):
    key = ("nc", mt_ets, debug)
    if key not in _CACHE:
        _CACHE[key] = _build(mt_ets, debug)
    return _CACHE[key]


def _prep_in_maps(inputs):
    import ml_dtypes
    bf16 = ml_dtypes.bfloat16
    f8 = ml_dtypes.float8_e4m3

    att = np.asarray(inputs["attention"], np.float32)          # [16, 2048, 2048]
    seq = np.asarray(inputs["sequence_output"], np.float32)
    mention_idx = np.asarray(inputs["mention_idx"], np.int32)  # [1024]
    entity_ids = np.asarray(inputs["entity_ids"], np.int32)    # [1024]
    pair_h = np.asarray(inputs["pair_h"], np.int32)            # [2048]
    pair_t = np.asarray(inputs["pair_t"], np.int32)

    counts = np.bincount(entity_ids, minlength=E).astype(np.float32)
    inv_cnt = 1.0 / np.maximum(counts, 1.0)

    ohe = np.zeros((NM, E), np.float32)
    ohe[np.arange(NM), entity_ids] = 1.0
    ohm = np.zeros((NM, E), np.float32)
    ohm[np.arange(NM), entity_ids] = inv_cnt[entity_ids]
    has0r = (counts == 0).astype(np.float32)[None, :]

    # which entity-128-halves each mention tile touches (all-zero slabs skipped)
    mt_ets = tuple(
        tuple(sorted(set((entity_ids[mt * 128:(mt + 1) * 128] // 128).tolist())))
        for mt in range(NMT))

    m_off = mention_idx.reshape(NMT, 128).T.copy()             # [128, 8]

    order = np.argsort(pair_h, kind="stable")
    sph = pair_h[order]
    spt = pair_t[order]
    p_off = np.zeros((128, 2 * NPT), np.int32)
    for pt in range(NPT):
        seg = slice(pt * 128, (pt + 1) * 128)
        p_off[:, 2 * pt] = sph[seg]
        p_off[:, 2 * pt + 1] = spt[seg]

    att8 = att.astype(f8)                                      # [16, 2048, 2048]

    shared = {
        "seqb": seq.astype(bf16),
        "m_off": m_off,
        "p_off": p_off,
        "ohe": ohe.astype(bf16),
        "ohm": ohm.astype(f8),
        "has0r": has0r,
        "w_head": np.asarray(inputs["W_head"], np.float32).astype(bf16),
        "w_tail": np.asarray(inputs["W_tail"], np.float32).astype(bf16),
        "w_ctx": np.asarray(inputs["W_ctx"], np.float32).astype(bf16),
        "w_bil": np.asarray(inputs["W_bil"], np.float32).astype(bf16),
        "b_head": np.asarray(inputs["b_head"], np.float32).reshape(PH // 128, 128).T.copy(),
        "b_tail": np.asarray(inputs["b_tail"], np.float32).reshape(PH // 128, 128).T.copy(),
        "b_bil": np.asarray(inputs["b_bil"], np.float32).reshape(1, 1),
    }

    in_maps = []
    for k in range(NC):
        sk = k * SL
        att_kk = np.ascontiguousarray(
            att8[:, :, sk:sk + SL].transpose(1, 0, 2)).reshape(S, HS)
        ohh_kk = np.zeros((E, PL), np.float32)
        ohh_kk[sph[k * PL:(k + 1) * PL], np.arange(PL)] = 1.0
        oht_kk = np.zeros((E, PL), np.float32)
        oht_kk[spt[k * PL:(k + 1) * PL], np.arange(PL)] = 1.0
        m = dict(shared)
        m["att_k"] = att_kk
        m["ohh_k"] = ohh_kk.astype(bf16)
        m["oht_k"] = oht_kk.astype(bf16)
        in_maps.append(m)
    return in_maps, mt_ets


def _run(inputs, trace=False, debug=False):
    _ensure_axon_profile_hook()
    from concourse.bass_utils import run_bass_kernel_spmd
    in_maps, mt_ets = _prep_in_maps(inputs)
    nc = _get_nc(mt_ets, debug)
    res = run_bass_kernel_spmd(nc, in_maps, list(range(NC)), trace=trace)
    sorted_logits = np.concatenate([np.asarray(res.results[k]["out"][0], np.float32)
                                    for k in range(NC)])
    order = np.argsort(np.asarray(inputs["pair_h"], np.int32), kind="stable")
    logits = np.empty(P, np.float32)
    logits[order] = sorted_logits
    return logits, res


def kernel(**inputs) -> np.ndarray:
    logits, _ = _run(inputs, trace=False)
    return logits


def kernel_traced(**inputs):
    logits, res = _run(inputs, trace=True)
    return logits, res


def kernel_debug(**inputs):
    logits, res = _run(inputs, trace=False, debug=True)
    return logits, res
